# revision 16
# baseline (speedup 1.0000x reference)
"""MoE (8 experts, top-2) Trainium2 kernel, expert-parallel across 8 NeuronCores.

Strategy:
  - Each core owns one expert (weights sharded along the expert axis; gate
    replicated). Everything data-dependent runs on device:
      * router logits (fp32 matmul), top-2 + renormalized gate weights (DVE)
      * per-expert token compaction (gpsimd sparse_gather)
      * token dispatch (indirect DMA gather of selected token rows)
      * expert MLP GEMMs in fp32r (silu(x@w1) * (x@w3)) @ w2, scaled by the
        gate coefficient
  - Each core returns its expert's (transposed) token outputs + the compacted
    token index list; the host scatter-adds the 8 partial outputs (the
    "combine" / unshard step).
"""
import sys

sys.path.insert(0, "/opt/trn_rl_repo")

import numpy as np

T, H, II, E = 2048, 1024, 4096, 8
P = 128
NT = T // P          # 16 token tiles
HC = H // P          # 8 hidden chunks
IC = II // P         # 32 intermediate chunks
NH = 2               # pipelined token slices
TH = T // NH
NTH = TH // P        # token tiles per slice
NCORES = 8

_build_cache = {}


def _build(cap_h):
    """Build + schedule the per-core Tile kernel; cap_h = per-slice capacity."""
    import concourse.bass as bass
    import concourse.bacc as bacc
    import concourse.mybir as mybir
    from concourse.tile import TileContext

    f32 = mybir.dt.float32
    f32r = mybir.dt.float32r
    i32 = mybir.dt.int32
    u32 = mybir.dt.uint32
    u8 = mybir.dt.uint8
    bf16 = mybir.dt.bfloat16
    AF = mybir.ActivationFunctionType
    OP = mybir.AluOpType

    assert cap_h % 16 == 0 and cap_h <= 512
    cap = NH * cap_h
    cf = cap_h // 16     # free cols of [16, cf] compacted layout

    nc = bacc.Bacc("TRN2", target_bir_lowering=False)

    # ---- I/O ----
    xth = nc.declare_dram_parameter("xth", [H, T], bf16, isOutput=False)
    xtl = nc.declare_dram_parameter("xtl", [H, T], bf16, isOutput=False)
    x = nc.declare_dram_parameter("x", [T, H], bf16, isOutput=False)
    gwh = nc.declare_dram_parameter("gwh", [H, E], bf16, isOutput=False)
    gwl = nc.declare_dram_parameter("gwl", [H, E], bf16, isOutput=False)
    w1 = nc.declare_dram_parameter("w1", [H, II], bf16, isOutput=False)
    w3 = nc.declare_dram_parameter("w3", [H, II], bf16, isOutput=False)
    w2 = nc.declare_dram_parameter("w2", [II, H], bf16, isOutput=False)
    oh = nc.declare_dram_parameter("oh", [P, NT * E], f32, isOutput=False)
    tokid = nc.declare_dram_parameter("tokid", [P, NT], f32, isOutput=False)
    slotg_d = nc.declare_dram_parameter("slotg", [16, cf], f32, isOutput=False)
    ident = nc.declare_dram_parameter("ident", [P, P], f32, isOutput=False)

    o_yt = nc.declare_dram_parameter("o_yt", [H, cap], f32, isOutput=True)
    o_idx = nc.declare_dram_parameter("o_idx", [cap], i32, isOutput=True)
    o_cnt = nc.declare_dram_parameter("o_cnt", [1, NH], u32, isOutput=True)

    d_cf = [nc.dram_tensor(f"d_cf{h}", [cap_h], f32) for h in range(NH)]

    with TileContext(nc) as tc:
        with (
            tc.tile_pool(name="sb", bufs=1) as sb,
            tc.tile_pool(name="sbw", bufs=2) as sbw,
            tc.tile_pool(name="psum", bufs=2, space="PSUM") as psg,
        ):
            # ---- constants ----
            idt = sb.tile([P, P], f32, tag="idt")
            nc.sync.dma_start(out=idt[:], in_=ident[:])
            idtb = sb.tile([P, P], bf16, tag="idtb")
            nc.vector.tensor_copy(out=idtb[:], in_=idt[:])
            oh_sb = sb.tile([P, NT * E], f32, tag="oh")
            nc.sync.dma_start(out=oh_sb[:], in_=oh[:])
            tk = sb.tile([P, NT], f32, tag="tk")
            nc.sync.dma_start(out=tk[:], in_=tokid[:])
            slotg = sb.tile([16, cf], f32, tag="slotg")
            nc.sync.dma_start(out=slotg[:], in_=slotg_d[:])
            gw_h = sb.tile([P, HC * E], bf16, tag="gwh")
            nc.sync.dma_start(
                out=gw_h[:].rearrange("p (hc e) -> p hc e", e=E),
                in_=gwh[:].rearrange("(hc p) e -> p hc e", p=P),
            )
            gw_l = sb.tile([P, HC * E], bf16, tag="gwl")
            nc.sync.dma_start(
                out=gw_l[:].rearrange("p (hc e) -> p hc e", e=E),
                in_=gwl[:].rearrange("(hc p) e -> p hc e", p=P),
            )

            xgT = [[sb.tile([P, cap_h], bf16, tag=f"xgT{h}_{hc}", name=f"xgT{h}_{hc}")
                    for hc in range(HC)] for h in range(NH)]
            cbc = [sb.tile([P, cap_h], f32, tag=f"cbc{h}", name=f"cbc{h}") for h in range(NH)]
            actT = [[sb.tile([P, cap_h], bf16, tag=f"actT{h}_{ic}", name=f"actT{h}_{ic}")
                     for ic in range(IC)] for h in range(NH)]
            logitsT = sb.tile([E, T], f32, tag="logitsT")
            ones16 = sb.tile([1, 16], f32, tag="ones16")
            nc.vector.memset(ones16[:], 1.0)
            onesP = sb.tile([1, P], f32, tag="onesP")
            nc.vector.memset(onesP[:], 1.0)

            nft = cap_h // P
            rem = cap_h - nft * P
            gtiles = [(k * P, P) for k in range(nft)] + ([(nft * P, rem)] if rem else [])
            state = [dict() for _ in range(NH)]

            def emit_router(h):
                ps_l = [psg.tile([E, 512], f32, tag=f"mm{ngl}", name=f"psl{h}{ngl}")
                        for ngl in range(2)]
                for hc in range(HC):
                    xt_h = sbw.tile([P, TH], bf16, tag="xth", name=f"xth{h}{hc}", bufs=3)
                    nc.sync.dma_start(out=xt_h[:], in_=xth[hc * P:(hc + 1) * P, h * TH:(h + 1) * TH])
                    xt_l = sbw.tile([P, TH], bf16, tag="xtl", name=f"xtl{h}{hc}", bufs=3)
                    nc.sync.dma_start(out=xt_l[:], in_=xtl[hc * P:(hc + 1) * P, h * TH:(h + 1) * TH])
                    for ngl in range(2):
                        terms = [
                            (gw_h[:, hc * E:(hc + 1) * E], xt_h),
                            (gw_l[:, hc * E:(hc + 1) * E], xt_h),
                            (gw_h[:, hc * E:(hc + 1) * E], xt_l),
                        ]
                        for ti, (lw, xr) in enumerate(terms):
                            nc.tensor.matmul(
                                out=ps_l[ngl][:],
                                lhsT=lw,
                                rhs=xr[:, ngl * 512:(ngl + 1) * 512],
                                start=(hc == 0 and ti == 0),
                                stop=(hc == HC - 1 and ti == 2),
                            )
                for ngl in range(2):
                    nc.vector.tensor_copy(
                        out=logitsT[:, h * TH + ngl * 512:h * TH + (ngl + 1) * 512],
                        in_=ps_l[ngl][:],
                    )

            def emit_ltr(h):
                l_h = sb.tile([P, NTH * E], f32, tag=f"l{h}", name=f"l{h}")
                state[h]["l_h"] = l_h
                for ci in range(NTH):
                    tp = psg.tile([P, E], f32, tag="mm3", name=f"ltr{h}{ci}")
                    nc.tensor.transpose(
                        out=tp[:],
                        in_=logitsT[:, h * TH + ci * P:h * TH + (ci + 1) * P],
                        identity=idt[0:E, 0:E],
                    )
                    nc.vector.tensor_copy(out=l_h[:, ci * E:(ci + 1) * E], in_=tp[:])

            def emit_routing_dve(h):
                l3 = state[h]["l_h"][:].rearrange("p (t e) -> p t e", e=E)
                m1a = sb.tile([P, NTH, 4], f32, tag="m1a", name=f"m1a{h}")
                m2a = sb.tile([P, NTH, 4], f32, tag="m2a", name=f"m2a{h}")
                nc.vector.tensor_tensor(out=m1a[:], in0=l3[:, :, 0::2], in1=l3[:, :, 1::2], op=OP.max)
                nc.vector.tensor_tensor(out=m2a[:], in0=l3[:, :, 0::2], in1=l3[:, :, 1::2], op=OP.min)
                m1b = sb.tile([P, NTH, 2], f32, tag="m1b", name=f"m1b{h}")
                m2b = sb.tile([P, NTH, 2], f32, tag="m2b", name=f"m2b{h}")
                tmin = sb.tile([P, NTH, 2], f32, tag="tmin", name=f"tmin{h}")
                nc.vector.tensor_tensor(out=m1b[:], in0=m1a[:, :, 0::2], in1=m1a[:, :, 1::2], op=OP.max)
                nc.vector.tensor_tensor(out=tmin[:], in0=m1a[:, :, 0::2], in1=m1a[:, :, 1::2], op=OP.min)
                nc.vector.tensor_tensor(out=m2b[:], in0=m2a[:, :, 0::2], in1=m2a[:, :, 1::2], op=OP.max)
                nc.vector.tensor_tensor(out=m2b[:], in0=m2b[:], in1=tmin[:], op=OP.max)
                m1 = sb.tile([P, NTH, 1], f32, tag="m1", name=f"m1{h}")
                m2 = sb.tile([P, NTH, 1], f32, tag="m2", name=f"m2{h}")
                tmin2 = sb.tile([P, NTH, 1], f32, tag="tmin2", name=f"tmin2{h}")
                nc.vector.tensor_tensor(out=m1[:], in0=m1b[:, :, 0:1], in1=m1b[:, :, 1:2], op=OP.max)
                nc.vector.tensor_tensor(out=tmin2[:], in0=m1b[:, :, 0:1], in1=m1b[:, :, 1:2], op=OP.min)
                nc.vector.tensor_tensor(out=m2[:], in0=m2b[:, :, 0:1], in1=m2b[:, :, 1:2], op=OP.max)
                nc.vector.tensor_tensor(out=m2[:], in0=m2[:], in1=tmin2[:], op=OP.max)

                dq = sb.tile([P, NTH], f32, tag="dq", name=f"dq{h}")
                nc.vector.tensor_sub(out=dq[:], in0=m2[:, :, 0], in1=m1[:, :, 0])
                q = sb.tile([P, NTH], f32, tag="q", name=f"q{h}")
                nc.scalar.activation(out=q[:], in_=dq[:], func=AF.Exp)
                s = sb.tile([P, NTH], f32, tag="s", name=f"s{h}")
                nc.vector.tensor_scalar_add(s[:], q[:], 1.0)
                wt1 = sb.tile([P, NTH], f32, tag="wt1", name=f"wt1{h}")
                nc.vector.reciprocal(wt1[:], s[:])
                wt2 = sb.tile([P, NTH], f32, tag="wt2", name=f"wt2{h}")
                nc.vector.tensor_mul(out=wt2[:], in0=q[:], in1=wt1[:])

                le_m = sb.tile([P, NTH, E], f32, tag="lem", name=f"lem{h}")
                nc.vector.tensor_mul(
                    out=le_m[:], in0=l3[:],
                    in1=oh_sb[:, h * NTH * E:(h + 1) * NTH * E].rearrange("p (t e) -> p t e", e=E),
                )
                le = sb.tile([P, NTH], f32, tag="le", name=f"le{h}")
                nc.vector.reduce_sum(
                    out=le[:].rearrange("p (t o) -> p t o", o=1),
                    in_=le_m[:], axis=mybir.AxisListType.X,
                )

                eq1 = sb.tile([P, NTH], f32, tag="eq1", name=f"eq1{h}")
                eq2 = sb.tile([P, NTH], f32, tag="eq2", name=f"eq2{h}")
                nc.vector.tensor_tensor(out=eq1[:], in0=le[:], in1=m1[:, :, 0], op=OP.is_equal)
                nc.vector.tensor_tensor(out=eq2[:], in0=le[:], in1=m2[:, :, 0], op=OP.is_equal)
                coef = sb.tile([P, NTH], f32, tag="coef", name=f"coef{h}")
                t1 = sb.tile([P, NTH], f32, tag="t1", name=f"t1{h}")
                nc.vector.tensor_mul(out=coef[:], in0=eq1[:], in1=wt1[:])
                nc.vector.tensor_mul(out=t1[:], in0=eq2[:], in1=wt2[:])
                nc.vector.tensor_add(out=coef[:], in0=coef[:], in1=t1[:])
                selm = sb.tile([P, NTH], f32, tag="selm", name=f"selm{h}")
                nc.vector.tensor_add(out=selm[:], in0=eq1[:], in1=eq2[:])

                enc = sb.tile([P, 16], f32, tag="enc", name=f"enc{h}")
                nc.vector.memset(enc[:], -1.0)
                tkp = sb.tile([P, NTH], f32, tag="tkp", name=f"tkp{h}")
                nc.vector.tensor_scalar_add(tkp[:], tk[:, h * NTH:(h + 1) * NTH], 1.0)
                cfh = sb.tile([P, NTH], f32, tag="cfh", name=f"cfh{h}")
                nc.vector.tensor_scalar(cfh[:], coef[:], 0.999, 0.5, op0=OP.min, op1=OP.mult)
                vv = sb.tile([P, NTH], f32, tag="vv", name=f"vv{h}")
                nc.vector.tensor_add(out=vv[:], in0=tkp[:], in1=cfh[:])
                nc.vector.tensor_mul(out=vv[:], in0=vv[:], in1=selm[:])
                nc.vector.tensor_scalar_sub(vv[:], vv[:], 1.0)
                nc.vector.tensor_copy(out=enc[:, 0:NTH], in_=vv[:])
                state[h]["enc"] = enc

            def emit_compact(h):
                enc = state[h]["enc"]
                enc_t = sb.tile([16, P], f32, tag="enc_t", name=f"enc_t{h}")
                tp1 = psg.tile([16, P], f32, tag="mm3", name=f"etr{h}")
                nc.tensor.transpose(out=tp1[:], in_=enc[:], identity=idt[:])
                nc.vector.tensor_copy(out=enc_t[:], in_=tp1[:])

                sg_v = sb.tile([16, P], f32, tag="sgv", name=f"sgv{h}")
                nf1 = sb.tile([1, 1], u32, tag="nf1", name=f"nf1{h}")
                nc.gpsimd.sparse_gather(out=sg_v[:], in_=enc_t[:], num_found=nf1[:])
                nc.sync.dma_start(out=o_cnt[:, h:h + 1], in_=nf1[:])

                nf_f = sb.tile([1, 1], f32, tag="nff", name=f"nff{h}")
                nc.vector.tensor_copy(out=nf_f[:], in_=nf1[:])
                nf_b_ps = psg.tile([16, 1], f32, tag="mm3", name=f"nfb{h}")
                nc.tensor.matmul(out=nf_b_ps[:], lhsT=ones16[:], rhs=nf_f[:], start=True, stop=True)
                nf_b = sb.tile([16, 1], f32, tag="nfbs", name=f"nfbs{h}")
                nc.vector.tensor_copy(out=nf_b[:], in_=nf_b_ps[:])
                slot_mask = sb.tile([16, cf], u8, tag="slotm", name=f"slotm{h}")
                nc.vector.tensor_tensor(
                    out=slot_mask[:], in0=slotg[:],
                    in1=nf_b[:].to_broadcast([16, cf]), op=OP.is_lt,
                )
                v_f = sb.tile([16, cf], f32, tag="vf", name=f"vf{h}")
                nc.vector.memset(v_f[:], 0.0)
                nc.vector.copy_predicated(out=v_f[:], mask=slot_mask[:], data=sg_v[:, 0:cf])
                idx_i = sb.tile([16, cf], i32, tag="idxi", name=f"idxi{h}")
                nc.vector.tensor_copy(out=idx_i[:], in_=v_f[:])
                nc.sync.dma_start(
                    out=o_idx[h * cap_h:(h + 1) * cap_h].rearrange("(f p) -> p f", p=16),
                    in_=idx_i[:],
                )

                nc.sync.dma_start(out=d_cf[h][:].rearrange("(f p) -> p f", p=16), in_=v_f[:])
                v_sb = sb.tile([P, nft], f32, tag="vsb", name=f"vsb{h}")
                nc.sync.dma_start(out=v_sb[:], in_=d_cf[h][0:nft * P].rearrange("(k p) -> p k", p=P))
                idx_sb = sb.tile([P, nft], i32, tag="idxsb", name=f"idxsb{h}")
                nc.vector.tensor_copy(out=idx_sb[:], in_=v_sb[:])
                state[h]["idx_sb"] = idx_sb
                if rem:
                    v_sb2 = sb.tile([rem, 1], f32, tag="vsb2", name=f"vsb2{h}")
                    nc.sync.dma_start(
                        out=v_sb2[:],
                        in_=d_cf[h][nft * P:cap_h].rearrange("(p o) -> p o", o=1),
                    )
                    idx_sb2 = sb.tile([rem, 1], i32, tag="idxsb2", name=f"idxsb2{h}")
                    nc.vector.tensor_copy(out=idx_sb2[:], in_=v_sb2[:])
                    state[h]["idx_sb2"] = idx_sb2

                vrow = sb.tile([1, cap_h], f32, tag="vrow", name=f"vrow{h}")
                nc.sync.dma_start(out=vrow[:], in_=d_cf[h][:].rearrange("(o c) -> o c", o=1))
                vrow_i = sb.tile([1, cap_h], i32, tag="vrowi", name=f"vrowi{h}")
                nc.vector.tensor_copy(out=vrow_i[:], in_=vrow[:])
                vrow_f = sb.tile([1, cap_h], f32, tag="vrowf", name=f"vrowf{h}")
                nc.vector.tensor_copy(out=vrow_f[:], in_=vrow_i[:])
                cf_row = sb.tile([1, cap_h], f32, tag="cfrow", name=f"cfrow{h}")
                nc.vector.tensor_sub(out=cf_row[:], in0=vrow[:], in1=vrow_f[:])
                nc.vector.tensor_scalar_mul(cf_row[:], cf_row[:], 2.0)

                cb_ps = psg.tile([P, cap_h], f32, tag="mm2", name=f"cb{h}")
                nc.tensor.matmul(out=cb_ps[:], lhsT=onesP[:], rhs=cf_row[:], start=True, stop=True)
                nc.vector.tensor_copy(out=cbc[h][:], in_=cb_ps[:])

            def emit_gather_issue(h):
                xg_tiles = []
                for gi, (goff, gn) in enumerate(gtiles):
                    off_ap = (state[h]["idx_sb"][:, gi:gi + 1] if gn == P
                              else state[h]["idx_sb2"][:, 0:1])
                    xg = sbw.tile([gn, H], bf16, tag="xg", name=f"xg{h}{gi}", bufs=2 * len(gtiles))
                    nc.gpsimd.indirect_dma_start(
                        out=xg[:], out_offset=None,
                        in_=x[:],
                        in_offset=bass.IndirectOffsetOnAxis(ap=off_ap, axis=0),
                    )
                    xg_tiles.append(xg)
                state[h]["xg_tiles"] = xg_tiles

            def emit_xgtr(h):
                for gi, (goff, gn) in enumerate(gtiles):
                    xg = state[h]["xg_tiles"][gi]
                    for hc in range(HC):
                        tpx = psg.tile([P, gn], bf16, tag="mm3", name=f"xtr{h}{gi}{hc}")
                        nc.tensor.transpose(
                            out=tpx[:], in_=xg[:, hc * P:(hc + 1) * P],
                            identity=idtb[0:gn, 0:gn],
                        )
                        nc.vector.tensor_copy(out=xgT[h][hc][:, goff:goff + gn], in_=tpx[:])

            def emit_G(h, ics):
                for ic in ics:
                    w1_sl = sbw.tile([P, H], bf16, tag="w1sl", name=f"w1sl{h}{ic}")
                    nc.sync.dma_start(
                        out=w1_sl[:].rearrange("p (hc i) -> p hc i", i=P),
                        in_=w1[:, ic * P:(ic + 1) * P].rearrange("(hc p) i -> p hc i", p=P),
                    )
                    w3_sl = sbw.tile([P, H], bf16, tag="w3sl", name=f"w3sl{h}{ic}")
                    nc.sync.dma_start(
                        out=w3_sl[:].rearrange("p (hc i) -> p hc i", i=P),
                        in_=w3[:, ic * P:(ic + 1) * P].rearrange("(hc p) i -> p hc i", p=P),
                    )
                    ps1 = psg.tile([P, cap_h], f32, tag="mm0", name=f"ps1_{h}{ic}")
                    ps3 = psg.tile([P, cap_h], f32, tag="mm1", name=f"ps3_{h}{ic}")
                    for hc in range(HC):
                        nc.tensor.matmul(
                            out=ps1[:], lhsT=w1_sl[:, hc * P:(hc + 1) * P],
                            rhs=xgT[h][hc][:],
                            start=(hc == 0), stop=(hc == HC - 1),
                        )
                    for hc in range(HC):
                        nc.tensor.matmul(
                            out=ps3[:], lhsT=w3_sl[:, hc * P:(hc + 1) * P],
                            rhs=xgT[h][hc][:],
                            start=(hc == 0), stop=(hc == HC - 1),
                        )
                    sl = sbw.tile([P, cap_h], f32, tag="silu", name=f"silu{h}{ic}")
                    nc.scalar.activation(out=sl[:], in_=ps1[:], func=AF.Silu)
                    nc.vector.tensor_mul(out=actT[h][ic][:], in0=sl[:], in1=ps3[:])

            def emit_H():
                for hc in range(HC):
                    w2_sl = sbw.tile([P, II], bf16, tag="w2sl", name=f"w2sl{hc}")
                    nc.sync.dma_start(
                        out=w2_sl[:].rearrange("p (ic h) -> p ic h", h=P),
                        in_=w2[:, hc * P:(hc + 1) * P].rearrange("(ic p) h -> p ic h", p=P),
                    )
                    for h in range(NH):
                        pso = psg.tile([P, cap_h], f32, tag="mm2", name=f"pso{hc}{h}")
                        for ic in range(IC):
                            nc.tensor.matmul(
                                out=pso[:], lhsT=w2_sl[:, ic * P:(ic + 1) * P],
                                rhs=actT[h][ic][:],
                                start=(ic == 0), stop=(ic == IC - 1),
                            )
                        yt_sb = sbw.tile([P, cap_h], f32, tag="yt", name=f"yt{hc}{h}")
                        nc.vector.tensor_mul(out=yt_sb[:], in0=pso[:], in1=cbc[h][:])
                        nc.sync.dma_start(
                            out=o_yt[hc * P:(hc + 1) * P, h * cap_h:(h + 1) * cap_h],
                            in_=yt_sb[:],
                        )

            # ---- pipelined emission ----
            emit_router(0)
            emit_ltr(0)
            emit_router(1)
            emit_routing_dve(0)
            emit_compact(0)
            emit_gather_issue(0)
            emit_xgtr(0)
            emit_ltr(1)
            emit_routing_dve(1)
            emit_compact(1)
            emit_gather_issue(1)
            emit_G(0, range(0, 6))
            emit_xgtr(1)
            emit_G(0, range(6, IC))
            emit_G(1, range(IC))
            emit_H()

    nc.compile()
    return nc


def _get_built(cap_h):
    if cap_h not in _build_cache:
        _build_cache[cap_h] = _build(cap_h)
    return _build_cache[cap_h]


def _run(cap_h, hs, gate_w, w1s, w2s, w3s, trace=False):
    import ml_dtypes
    from concourse.bass_utils import run_bass_kernel_spmd

    nc = _get_built(cap_h)

    bf = ml_dtypes.bfloat16
    x_hi = hs.astype(bf)
    x_lo = (hs - x_hi.astype(np.float32)).astype(bf)
    xth_np = np.ascontiguousarray(x_hi.T)
    xtl_np = np.ascontiguousarray(x_lo.T)
    gw_hi = gate_w.astype(bf)
    gw_lo = (gate_w - gw_hi.astype(np.float32)).astype(bf)
    x_bf = np.ascontiguousarray(x_hi)
    oh_base = np.zeros((P, NT, E), np.float32)
    tokid_np = (np.arange(NT)[None, :] * P + np.arange(P)[:, None]).astype(np.float32)
    slotg_np = (np.arange(cap_h // 16)[None, :] * 16 + np.arange(16)[:, None]).astype(np.float32)
    ident_np = np.eye(P, dtype=np.float32)

    in_maps = []
    for c in range(NCORES):
        oh_c = oh_base.copy()
        oh_c[:, :, c] = 1.0
        in_maps.append({
            "xth": xth_np,
            "xtl": xtl_np,
            "x": x_bf,
            "gwh": gw_hi,
            "gwl": gw_lo,
            "w1": np.ascontiguousarray(w1s[c].astype(bf)),
            "w3": np.ascontiguousarray(w3s[c].astype(bf)),
            "w2": np.ascontiguousarray(w2s[c].astype(bf)),
            "oh": oh_c.reshape(P, NT * E),
            "tokid": tokid_np,
            "slotg": slotg_np,
            "ident": ident_np,
        })

    res = run_bass_kernel_spmd(nc, in_maps, list(range(NCORES)), trace=trace)
    return res


def kernel(hidden_states, gate_w, w1s, w2s, w3s, _trace=False, _cap_h=304):
    hs = np.ascontiguousarray(np.asarray(hidden_states, dtype=np.float32))
    gate_w = np.ascontiguousarray(np.asarray(gate_w, dtype=np.float32))
    w1s = np.asarray(w1s, dtype=np.float32)
    w2s = np.asarray(w2s, dtype=np.float32)
    w3s = np.asarray(w3s, dtype=np.float32)

    cap_h = _cap_h
    while True:
        res = _run(cap_h, hs, gate_w, w1s, w2s, w3s, trace=_trace)
        counts = [
            [int(res.results[c]["o_cnt"].ravel()[h]) for h in range(NH)]
            for c in range(NCORES)
        ]
        if max(max(cc) for cc in counts) <= cap_h:
            break
        if cap_h >= 512:
            raise RuntimeError("token slice exceeded 512-slot capacity")
        cap_h = 512  # capacity overflow fallback (recompiles; correctness kept)

    out = np.zeros((T, H), dtype=np.float32)
    for c in range(NCORES):
        r = res.results[c]
        y = np.ascontiguousarray(r["o_yt"].T)
        for h in range(NH):
            cnt = counts[c][h]
            idx = r["o_idx"][h * cap_h:h * cap_h + cnt]
            out[idx] += y[h * cap_h:h * cap_h + cnt]
    kernel._last_results = res
    return out


# revision 18
# speedup vs baseline: 1.1839x; 1.1839x over previous
"""MoE (8 experts, top-2) Trainium2 kernel, expert-parallel across 8 NeuronCores.

Strategy:
  - Each core owns one expert (weights sharded along the expert axis; gate
    replicated). Everything data-dependent runs on device:
      * router logits (fp32 matmul), top-2 + renormalized gate weights (DVE)
      * per-expert token compaction (gpsimd sparse_gather)
      * token dispatch (indirect DMA gather of selected token rows)
      * expert MLP GEMMs in fp32r (silu(x@w1) * (x@w3)) @ w2, scaled by the
        gate coefficient
  - Each core returns its expert's (transposed) token outputs + the compacted
    token index list; the host scatter-adds the 8 partial outputs (the
    "combine" / unshard step).
"""
import sys

sys.path.insert(0, "/opt/trn_rl_repo")

import numpy as np

T, H, II, E = 2048, 1024, 4096, 8
P = 128
NT = T // P          # 16 token tiles
HC = H // P          # 8 hidden chunks
IC = II // P         # 32 intermediate chunks
NCORES = 8

_build_cache = {}


def _build(cap):
    """Build + schedule the per-core Tile kernel for token capacity `cap`."""
    import concourse.bass as bass
    import concourse.bacc as bacc
    import concourse.mybir as mybir
    from concourse.tile import TileContext

    f32 = mybir.dt.float32
    f32r = mybir.dt.float32r
    i32 = mybir.dt.int32
    u32 = mybir.dt.uint32
    u8 = mybir.dt.uint8
    bf16 = mybir.dt.bfloat16
    AF = mybir.ActivationFunctionType
    OP = mybir.AluOpType

    _grp_table = {512: 2, 608: 2, 640: 2, 768: 2, 896: 2, 1024: 2, 1536: 3, 2048: 4}
    assert cap in _grp_table, cap
    ngrp = _grp_table[cap]
    grp = cap // ngrp    # token group size per PSUM accumulation (<=512, >=256)
    cf = cap // 16       # free cols of [16, cf] compacted layout
    ntt = cap // P       # gather tiles

    nc = bacc.Bacc("TRN2", target_bir_lowering=False)

    # ---- I/O ----
    xth = nc.declare_dram_parameter("xth", [H, T], bf16, isOutput=False)
    xtl = nc.declare_dram_parameter("xtl", [H, T], bf16, isOutput=False)
    x = nc.declare_dram_parameter("x", [T, H], bf16, isOutput=False)
    gwh = nc.declare_dram_parameter("gwh", [H, E], bf16, isOutput=False)
    gwl = nc.declare_dram_parameter("gwl", [H, E], bf16, isOutput=False)
    w1 = nc.declare_dram_parameter("w1", [H, II], bf16, isOutput=False)
    w3 = nc.declare_dram_parameter("w3", [H, II], bf16, isOutput=False)
    w2 = nc.declare_dram_parameter("w2", [II, H], bf16, isOutput=False)
    oh = nc.declare_dram_parameter("oh", [P, NT * E], f32, isOutput=False)
    tokid = nc.declare_dram_parameter("tokid", [P, NT], f32, isOutput=False)
    slotg_d = nc.declare_dram_parameter("slotg", [16, cf], f32, isOutput=False)
    ident = nc.declare_dram_parameter("ident", [P, P], f32, isOutput=False)

    o_yt = nc.declare_dram_parameter("o_yt", [H, cap], f32, isOutput=True)
    o_idx = nc.declare_dram_parameter("o_idx", [cap], i32, isOutput=True)
    o_cnt = nc.declare_dram_parameter("o_cnt", [1, 1], u32, isOutput=True)


    with TileContext(nc) as tc:
        with (
            tc.tile_pool(name="sb", bufs=1) as sb,
            tc.tile_pool(name="sbw", bufs=2) as sbw,
            tc.tile_pool(name="psum", bufs=2, space="PSUM") as psg,
            tc.tile_pool(name="drp", bufs=1, space="DRAM") as drp,
        ):
            d_cf = drp.tile([cap], f32, tag="d_cf")
            # ---- constants ----
            idt = sb.tile([P, P], f32, tag="idt")
            nc.sync.dma_start(out=idt[:], in_=ident[:])
            idtb = sb.tile([P, P], bf16, tag="idtb")
            nc.vector.tensor_copy(out=idtb[:], in_=idt[:])
            oh_sb = sb.tile([P, NT * E], f32, tag="oh")
            nc.sync.dma_start(out=oh_sb[:], in_=oh[:])
            tk = sb.tile([P, NT], f32, tag="tk")
            nc.sync.dma_start(out=tk[:], in_=tokid[:])
            slotg = sb.tile([16, cf], f32, tag="slotg")
            nc.sync.dma_start(out=slotg[:], in_=slotg_d[:])
            gw_h = sb.tile([P, HC * E], bf16, tag="gwh")
            nc.sync.dma_start(
                out=gw_h[:].rearrange("p (hc e) -> p hc e", e=E),
                in_=gwh[:].rearrange("(hc p) e -> p hc e", p=P),
            )
            gw_l = sb.tile([P, HC * E], bf16, tag="gwl")
            nc.sync.dma_start(
                out=gw_l[:].rearrange("p (hc e) -> p hc e", e=E),
                in_=gwl[:].rearrange("(hc p) e -> p hc e", p=P),
            )

            # ---- A. router: logitsT [8, 2048] = gw.T @ x.T, fp32 ----
            logitsT = sb.tile([E, T], f32, tag="logitsT")
            ps_l = [psg.tile([E, 512], f32, tag=f"mm{ng}", name=f"psl{ng}") for ng in range(4)]
            for hc in range(HC):
                xt_h = sbw.tile([P, T], bf16, tag="xth")
                nc.sync.dma_start(out=xt_h[:], in_=xth[hc * P:(hc + 1) * P, :])
                xt_l = sbw.tile([P, T], bf16, tag="xtl")
                nc.sync.dma_start(out=xt_l[:], in_=xtl[hc * P:(hc + 1) * P, :])
                for ng in range(4):
                    terms = [
                        (gw_h[:, hc * E:(hc + 1) * E], xt_h),
                        (gw_l[:, hc * E:(hc + 1) * E], xt_h),
                        (gw_h[:, hc * E:(hc + 1) * E], xt_l),
                    ]
                    for ti, (lw, xr) in enumerate(terms):
                        nc.tensor.matmul(
                            out=ps_l[ng][:],
                            lhsT=lw,
                            rhs=xr[:, ng * 512:(ng + 1) * 512],
                            start=(hc == 0 and ti == 0),
                            stop=(hc == HC - 1 and ti == 2),
                        )
            for ng in range(4):
                nc.vector.tensor_copy(
                    out=logitsT[:, ng * 512:(ng + 1) * 512], in_=ps_l[ng][:]
                )

            # ---- B. transpose logitsT -> l_all [128, (16, 8)] ----
            l_all = sb.tile([P, NT * E], f32, tag="l_all")
            for ci in range(NT):
                tp = psg.tile([P, E], f32, tag="mm3")
                nc.tensor.transpose(
                    out=tp[:],
                    in_=logitsT[:, ci * P:(ci + 1) * P],
                    identity=idt[0:E, 0:E],
                )
                nc.vector.tensor_copy(out=l_all[:, ci * E:(ci + 1) * E], in_=tp[:])

            # ---- C. top-2 + coef ----
            l3 = l_all[:].rearrange("p (t e) -> p t e", e=E)
            m1a = sb.tile([P, NT, 4], f32, tag="m1a")
            m2a = sb.tile([P, NT, 4], f32, tag="m2a")
            nc.vector.tensor_tensor(out=m1a[:], in0=l3[:, :, 0::2], in1=l3[:, :, 1::2], op=OP.max)
            nc.vector.tensor_tensor(out=m2a[:], in0=l3[:, :, 0::2], in1=l3[:, :, 1::2], op=OP.min)
            m1b = sb.tile([P, NT, 2], f32, tag="m1b")
            m2b = sb.tile([P, NT, 2], f32, tag="m2b")
            tmin = sb.tile([P, NT, 2], f32, tag="tmin")
            nc.vector.tensor_tensor(out=m1b[:], in0=m1a[:, :, 0::2], in1=m1a[:, :, 1::2], op=OP.max)
            nc.vector.tensor_tensor(out=tmin[:], in0=m1a[:, :, 0::2], in1=m1a[:, :, 1::2], op=OP.min)
            nc.vector.tensor_tensor(out=m2b[:], in0=m2a[:, :, 0::2], in1=m2a[:, :, 1::2], op=OP.max)
            nc.vector.tensor_tensor(out=m2b[:], in0=m2b[:], in1=tmin[:], op=OP.max)
            m1 = sb.tile([P, NT, 1], f32, tag="m1")
            m2 = sb.tile([P, NT, 1], f32, tag="m2")
            tmin2 = sb.tile([P, NT, 1], f32, tag="tmin2")
            nc.vector.tensor_tensor(out=m1[:], in0=m1b[:, :, 0:1], in1=m1b[:, :, 1:2], op=OP.max)
            nc.vector.tensor_tensor(out=tmin2[:], in0=m1b[:, :, 0:1], in1=m1b[:, :, 1:2], op=OP.min)
            nc.vector.tensor_tensor(out=m2[:], in0=m2b[:, :, 0:1], in1=m2b[:, :, 1:2], op=OP.max)
            nc.vector.tensor_tensor(out=m2[:], in0=m2[:], in1=tmin2[:], op=OP.max)

            dq = sb.tile([P, NT], f32, tag="dq")
            nc.vector.tensor_sub(out=dq[:], in0=m2[:, :, 0], in1=m1[:, :, 0])
            q = sb.tile([P, NT], f32, tag="q")
            nc.scalar.activation(out=q[:], in_=dq[:], func=AF.Exp)
            s = sb.tile([P, NT], f32, tag="s")
            nc.vector.tensor_scalar_add(s[:], q[:], 1.0)
            wt1 = sb.tile([P, NT], f32, tag="wt1")
            nc.vector.reciprocal(wt1[:], s[:])
            wt2 = sb.tile([P, NT], f32, tag="wt2")
            nc.vector.tensor_mul(out=wt2[:], in0=q[:], in1=wt1[:])

            le_m = sb.tile([P, NT, E], f32, tag="lem")
            nc.vector.tensor_mul(
                out=le_m[:], in0=l3[:], in1=oh_sb[:].rearrange("p (t e) -> p t e", e=E)
            )
            le = sb.tile([P, NT], f32, tag="le")
            nc.vector.reduce_sum(
                out=le[:].rearrange("p (t o) -> p t o", o=1),
                in_=le_m[:],
                axis=mybir.AxisListType.X,
            )

            eq1 = sb.tile([P, NT], f32, tag="eq1")
            eq2 = sb.tile([P, NT], f32, tag="eq2")
            nc.vector.tensor_tensor(out=eq1[:], in0=le[:], in1=m1[:, :, 0], op=OP.is_equal)
            nc.vector.tensor_tensor(out=eq2[:], in0=le[:], in1=m2[:, :, 0], op=OP.is_equal)
            coef = sb.tile([P, NT], f32, tag="coef")
            t1 = sb.tile([P, NT], f32, tag="t1")
            nc.vector.tensor_mul(out=coef[:], in0=eq1[:], in1=wt1[:])
            nc.vector.tensor_mul(out=t1[:], in0=eq2[:], in1=wt2[:])
            nc.vector.tensor_add(out=coef[:], in0=coef[:], in1=t1[:])
            selm = sb.tile([P, NT], f32, tag="selm")
            nc.vector.tensor_add(out=selm[:], in0=eq1[:], in1=eq2[:])

            # ---- D. compaction ----
            # pack token id + gate coef in one fp32: v = tokid + min(coef,.999)/2
            # (coef recovered as 2*frac(v); idx as round-to-nearest of v)
            enc = sb.tile([P, NT], f32, tag="enc")
            tkp = sb.tile([P, NT], f32, tag="tkp")
            nc.vector.tensor_scalar_add(tkp[:], tk[:], 1.0)
            cfh = sb.tile([P, NT], f32, tag="cfh")
            nc.vector.tensor_scalar(cfh[:], coef[:], 0.999, 0.5, op0=OP.min, op1=OP.mult)
            nc.vector.tensor_add(out=enc[:], in0=tkp[:], in1=cfh[:])
            nc.vector.tensor_mul(out=enc[:], in0=enc[:], in1=selm[:])
            nc.vector.tensor_scalar_sub(enc[:], enc[:], 1.0)

            enc_t = sb.tile([NT, P], f32, tag="enc_t")
            tp1 = psg.tile([NT, P], f32, tag="mm3")
            nc.tensor.transpose(out=tp1[:], in_=enc[:], identity=idt[:])
            nc.vector.tensor_copy(out=enc_t[:], in_=tp1[:])

            sg_v = sb.tile([16, P], f32, tag="sgv")
            nf1 = sb.tile([1, 1], u32, tag="nf1")
            nc.gpsimd.sparse_gather(out=sg_v[:], in_=enc_t[:], num_found=nf1[:])
            nc.sync.dma_start(out=o_cnt[:], in_=nf1[:])

            # valid-slot mask (sparse_gather tail is garbage on HW)
            nf_f = sb.tile([1, 1], f32, tag="nff")
            nc.vector.tensor_copy(out=nf_f[:], in_=nf1[:])
            ones16 = sb.tile([1, 16], f32, tag="ones16")
            nc.vector.memset(ones16[:], 1.0)
            nf_b_ps = psg.tile([16, 1], f32, tag="mm3")
            nc.tensor.matmul(out=nf_b_ps[:], lhsT=ones16[:], rhs=nf_f[:], start=True, stop=True)
            nf_b = sb.tile([16, 1], f32, tag="nfbs")
            nc.vector.tensor_copy(out=nf_b[:], in_=nf_b_ps[:])
            slot_mask = sb.tile([16, cf], u8, tag="slotm")
            nc.vector.tensor_tensor(
                out=slot_mask[:], in0=slotg[:],
                in1=nf_b[:].to_broadcast([16, cf]), op=OP.is_lt,
            )
            v_f = sb.tile([16, cf], f32, tag="vf")
            nc.vector.memset(v_f[:], 0.0)
            nc.vector.copy_predicated(out=v_f[:], mask=slot_mask[:], data=sg_v[:, 0:cf])
            idx_i = sb.tile([16, cf], i32, tag="idxi")
            nc.vector.tensor_copy(out=idx_i[:], in_=v_f[:])
            nc.sync.dma_start(out=o_idx[:].rearrange("(f p) -> p f", p=16), in_=idx_i[:])

            # free-major relayout of packed values through DRAM
            nc.sync.dma_start(out=d_cf[:].rearrange("(f p) -> p f", p=16), in_=v_f[:])
            nft = cap // P           # full 128-token gather tiles
            rem = cap - nft * P      # remainder tile rows
            v_sb = sb.tile([P, nft], f32, tag="vsb")
            nc.sync.dma_start(
                out=v_sb[:], in_=d_cf[0:nft * P].rearrange("(k p) -> p k", p=P)
            )
            idx_sb = sb.tile([P, nft], i32, tag="idxsb")
            nc.vector.tensor_copy(out=idx_sb[:], in_=v_sb[:])
            if rem:
                v_sb2 = sb.tile([rem, 1], f32, tag="vsb2")
                nc.sync.dma_start(
                    out=v_sb2[:],
                    in_=d_cf[nft * P:cap].rearrange("(p o) -> p o", o=1),
                )
                idx_sb2 = sb.tile([rem, 1], i32, tag="idxsb2")
                nc.vector.tensor_copy(out=idx_sb2[:], in_=v_sb2[:])
            vrow = sb.tile([1, cap], f32, tag="vrow")
            nc.sync.dma_start(out=vrow[:], in_=d_cf[:].rearrange("(o c) -> o c", o=1))
            vrow_i = sb.tile([1, cap], i32, tag="vrowi")
            nc.vector.tensor_copy(out=vrow_i[:], in_=vrow[:])
            vrow_f = sb.tile([1, cap], f32, tag="vrowf")
            nc.vector.tensor_copy(out=vrow_f[:], in_=vrow_i[:])
            cf_row = sb.tile([1, cap], f32, tag="cfrow")
            nc.vector.tensor_sub(out=cf_row[:], in0=vrow[:], in1=vrow_f[:])
            nc.vector.tensor_scalar_mul(cf_row[:], cf_row[:], 2.0)

            # ---- F. coef broadcast [128, cap] ----
            onesP = sb.tile([1, P], f32, tag="onesP")
            nc.vector.memset(onesP[:], 1.0)
            cbc = sb.tile([P, cap], f32, tag="cbc")
            for g in range(ngrp):
                cb_ps = psg.tile([P, grp], f32, tag="mm2")
                nc.tensor.matmul(
                    out=cb_ps[:], lhsT=onesP[:],
                    rhs=cf_row[:, g * grp:(g + 1) * grp], start=True, stop=True,
                )
                nc.vector.tensor_copy(out=cbc[:, g * grp:(g + 1) * grp], in_=cb_ps[:])

            # ---- E. gather selected token rows + transpose to [H, cap] ----
            xgT = [sb.tile([P, cap], bf16, tag=f"xgT{hc}", name=f"xgT{hc}") for hc in range(HC)]
            gtiles = [(k * P, P) for k in range(nft)] + ([(nft * P, rem)] if rem else [])
            for gi, (goff, gn) in enumerate(gtiles):
                off_ap = idx_sb[:, gi:gi + 1] if gn == P else idx_sb2[:, 0:1]
                xg = sbw.tile([gn, H], bf16, tag="xg", name=f"xg{gi}", bufs=3)
                nc.gpsimd.indirect_dma_start(
                    out=xg[:], out_offset=None,
                    in_=x[:],
                    in_offset=bass.IndirectOffsetOnAxis(ap=off_ap, axis=0),
                )
                for hc in range(HC):
                    tpx = psg.tile([P, gn], bf16, tag="mm3", name=f"xtr{gi}{hc}")
                    nc.tensor.transpose(
                        out=tpx[:], in_=xg[:, hc * P:(hc + 1) * P],
                        identity=idtb[0:gn, 0:gn],
                    )
                    nc.vector.tensor_copy(
                        out=xgT[hc][:, goff:goff + gn], in_=tpx[:]
                    )

            # ---- G. h1 = x@w1, h3 = x@w3 (transposed), fused silu*mul ----
            actT = [sb.tile([P, cap], bf16, tag=f"actT{ic}", name=f"actT{ic}") for ic in range(IC)]
            for ic in range(IC):
                w1_sl = sbw.tile([P, H], bf16, tag="w1sl")
                nc.sync.dma_start(
                    out=w1_sl[:].rearrange("p (hc i) -> p hc i", i=P),
                    in_=w1[:, ic * P:(ic + 1) * P].rearrange("(hc p) i -> p hc i", p=P),
                )
                w3_sl = sbw.tile([P, H], bf16, tag="w3sl")
                nc.sync.dma_start(
                    out=w3_sl[:].rearrange("p (hc i) -> p hc i", i=P),
                    in_=w3[:, ic * P:(ic + 1) * P].rearrange("(hc p) i -> p hc i", p=P),
                )
                for g in range(ngrp):
                    gs = slice(g * grp, (g + 1) * grp)
                    ps1 = psg.tile([P, grp], f32, tag="mm0")
                    ps3 = psg.tile([P, grp], f32, tag="mm1")
                    for hc in range(HC):
                        nc.tensor.matmul(
                            out=ps1[:],
                            lhsT=w1_sl[:, hc * P:(hc + 1) * P],
                            rhs=xgT[hc][:, gs],
                            start=(hc == 0), stop=(hc == HC - 1),
                        )
                    for hc in range(HC):
                        nc.tensor.matmul(
                            out=ps3[:],
                            lhsT=w3_sl[:, hc * P:(hc + 1) * P],
                            rhs=xgT[hc][:, gs],
                            start=(hc == 0), stop=(hc == HC - 1),
                        )
                    sl = sbw.tile([P, grp], f32, tag="silu")
                    nc.scalar.activation(out=sl[:], in_=ps1[:], func=AF.Silu)
                    nc.vector.tensor_mul(out=actT[ic][:, gs], in0=sl[:], in1=ps3[:])

            # ---- H. yT = (act @ w2).T * coef ----
            for hc in range(HC):
                w2_sl = sbw.tile([P, II], bf16, tag="w2sl")
                nc.sync.dma_start(
                    out=w2_sl[:].rearrange("p (ic h) -> p ic h", h=P),
                    in_=w2[:, hc * P:(hc + 1) * P].rearrange("(ic p) h -> p ic h", p=P),
                )
                for g in range(ngrp):
                    gs = slice(g * grp, (g + 1) * grp)
                    pso = psg.tile([P, grp], f32, tag="mm2")
                    for ic in range(IC):
                        nc.tensor.matmul(
                            out=pso[:],
                            lhsT=w2_sl[:, ic * P:(ic + 1) * P],
                            rhs=actT[ic][:, gs],
                            start=(ic == 0), stop=(ic == IC - 1),
                        )
                    yt_sb = sbw.tile([P, grp], f32, tag="yt")
                    nc.vector.tensor_mul(out=yt_sb[:], in0=pso[:], in1=cbc[:, gs])
                    nc.sync.dma_start(
                        out=o_yt[hc * P:(hc + 1) * P, gs], in_=yt_sb[:]
                    )

    nc.compile()
    return nc


def _get_built(cap):
    if cap not in _build_cache:
        _build_cache[cap] = _build(cap)
    return _build_cache[cap]


def _run(cap, hs, gate_w, w1s, w2s, w3s, trace=False):
    import ml_dtypes
    from concourse.bass_utils import run_bass_kernel_spmd

    nc = _get_built(cap)

    bf = ml_dtypes.bfloat16
    x_hi = hs.astype(bf)
    x_lo = (hs - x_hi.astype(np.float32)).astype(bf)
    xth_np = np.ascontiguousarray(x_hi.T)
    xtl_np = np.ascontiguousarray(x_lo.T)
    gw_hi = gate_w.astype(bf)
    gw_lo = (gate_w - gw_hi.astype(np.float32)).astype(bf)
    x_bf = np.ascontiguousarray(x_hi)
    oh_base = np.zeros((P, NT, E), np.float32)
    tokid_np = (np.arange(NT)[None, :] * P + np.arange(P)[:, None]).astype(np.float32)
    slotg_np = (np.arange(cap // 16)[None, :] * 16 + np.arange(16)[:, None]).astype(np.float32)
    ident_np = np.eye(P, dtype=np.float32)

    in_maps = []
    for c in range(NCORES):
        oh_c = oh_base.copy()
        oh_c[:, :, c] = 1.0
        in_maps.append({
            "xth": xth_np,
            "xtl": xtl_np,
            "x": x_bf,
            "gwh": gw_hi,
            "gwl": gw_lo,
            "w1": np.ascontiguousarray(w1s[c].astype(bf)),
            "w3": np.ascontiguousarray(w3s[c].astype(bf)),
            "w2": np.ascontiguousarray(w2s[c].astype(bf)),
            "oh": oh_c.reshape(P, NT * E),
            "tokid": tokid_np,
            "slotg": slotg_np,
            "ident": ident_np,
        })

    res = run_bass_kernel_spmd(nc, in_maps, list(range(NCORES)), trace=trace)
    return res


def kernel(hidden_states, gate_w, w1s, w2s, w3s, _trace=False, _cap=608):
    hs = np.ascontiguousarray(np.asarray(hidden_states, dtype=np.float32))
    gate_w = np.ascontiguousarray(np.asarray(gate_w, dtype=np.float32))
    w1s = np.asarray(w1s, dtype=np.float32)
    w2s = np.asarray(w2s, dtype=np.float32)
    w3s = np.asarray(w3s, dtype=np.float32)

    cap = _cap
    while True:
        res = _run(cap, hs, gate_w, w1s, w2s, w3s, trace=_trace)
        counts = [int(res.results[c]["o_cnt"].ravel()[0]) for c in range(NCORES)]
        if max(counts) <= cap:
            break
        # capacity overflow (won't happen for sane routing): rebuild bigger
        cap = 2048 if max(counts) > 1024 else 1024

    out = np.zeros((T, H), dtype=np.float32)
    for c in range(NCORES):
        r = res.results[c]
        cnt = counts[c]
        idx = r["o_idx"][:cnt]
        y = np.ascontiguousarray(r["o_yt"].T[:cnt])
        out[idx] += y
    kernel._last_results = res
    return out


# revision 19
# speedup vs baseline: 1.2070x; 1.0196x over previous
"""MoE (8 experts, top-2) Trainium2 kernel, expert-parallel across 8 NeuronCores.

Strategy:
  - Each core owns one expert (weights sharded along the expert axis; gate
    replicated). Everything data-dependent runs on device:
      * router logits (fp32 matmul), top-2 + renormalized gate weights (DVE)
      * per-expert token compaction (gpsimd sparse_gather)
      * token dispatch (indirect DMA gather of selected token rows)
      * expert MLP GEMMs in fp32r (silu(x@w1) * (x@w3)) @ w2, scaled by the
        gate coefficient
  - Each core returns its expert's (transposed) token outputs + the compacted
    token index list; the host scatter-adds the 8 partial outputs (the
    "combine" / unshard step).
"""
import sys

sys.path.insert(0, "/opt/trn_rl_repo")

import numpy as np

T, H, II, E = 2048, 1024, 4096, 8
P = 128
NT = T // P          # 16 token tiles
HC = H // P          # 8 hidden chunks
IC = II // P         # 32 intermediate chunks
NCORES = 8

_build_cache = {}


def _build(cap):
    """Build + schedule the per-core Tile kernel for token capacity `cap`."""
    import concourse.bass as bass
    import concourse.bacc as bacc
    import concourse.mybir as mybir
    from concourse.tile import TileContext

    f32 = mybir.dt.float32
    f32r = mybir.dt.float32r
    i32 = mybir.dt.int32
    u32 = mybir.dt.uint32
    u8 = mybir.dt.uint8
    bf16 = mybir.dt.bfloat16
    AF = mybir.ActivationFunctionType
    OP = mybir.AluOpType

    _grp_table = {512: 2, 608: 2, 640: 2, 768: 2, 896: 2, 1024: 2, 1536: 3, 2048: 4}
    assert cap in _grp_table, cap
    ngrp = _grp_table[cap]
    grp = cap // ngrp    # token group size per PSUM accumulation (<=512, >=256)
    cf = cap // 16       # free cols of [16, cf] compacted layout
    ntt = cap // P       # gather tiles

    nc = bacc.Bacc("TRN2", target_bir_lowering=False)

    # ---- I/O ----
    xth = nc.declare_dram_parameter("xth", [H, T], bf16, isOutput=False)
    xtl = nc.declare_dram_parameter("xtl", [H, T], bf16, isOutput=False)
    x = nc.declare_dram_parameter("x", [T, H], bf16, isOutput=False)
    gwh = nc.declare_dram_parameter("gwh", [H, E], bf16, isOutput=False)
    gwl = nc.declare_dram_parameter("gwl", [H, E], bf16, isOutput=False)
    w1 = nc.declare_dram_parameter("w1", [H, II], bf16, isOutput=False)
    w3 = nc.declare_dram_parameter("w3", [H, II], bf16, isOutput=False)
    w2 = nc.declare_dram_parameter("w2", [II, H], bf16, isOutput=False)
    oh = nc.declare_dram_parameter("oh", [P, NT * E], f32, isOutput=False)
    tokid = nc.declare_dram_parameter("tokid", [P, NT], f32, isOutput=False)
    slotg_d = nc.declare_dram_parameter("slotg", [16, cf], f32, isOutput=False)
    ident = nc.declare_dram_parameter("ident", [P, P], f32, isOutput=False)

    o_yt = nc.declare_dram_parameter("o_yt", [H, cap], f32, isOutput=True)
    o_idx = nc.declare_dram_parameter("o_idx", [cap], i32, isOutput=True)
    o_cnt = nc.declare_dram_parameter("o_cnt", [1, 1], u32, isOutput=True)


    with TileContext(nc) as tc:
        with (
            tc.tile_pool(name="sb", bufs=1) as sb,
            tc.tile_pool(name="sbw", bufs=2) as sbw,
            tc.tile_pool(name="psum", bufs=2, space="PSUM") as psg,
            tc.tile_pool(name="drp", bufs=1, space="DRAM") as drp,
        ):
            d_cf = drp.tile([cap], f32, tag="d_cf")
            # ---- constants ----
            idt = sb.tile([P, P], f32, tag="idt")
            nc.sync.dma_start(out=idt[:], in_=ident[:])
            idtb = sb.tile([P, P], bf16, tag="idtb")
            nc.vector.tensor_copy(out=idtb[:], in_=idt[:])
            oh_sb = sb.tile([P, NT * E], f32, tag="oh")
            nc.sync.dma_start(out=oh_sb[:], in_=oh[:])
            tk = sb.tile([P, NT], f32, tag="tk")
            nc.sync.dma_start(out=tk[:], in_=tokid[:])
            slotg = sb.tile([16, cf], f32, tag="slotg")
            nc.sync.dma_start(out=slotg[:], in_=slotg_d[:])
            gw_h = sb.tile([P, HC * E], bf16, tag="gwh")
            nc.sync.dma_start(
                out=gw_h[:].rearrange("p (hc e) -> p hc e", e=E),
                in_=gwh[:].rearrange("(hc p) e -> p hc e", p=P),
            )
            gw_l = sb.tile([P, HC * E], bf16, tag="gwl")
            nc.sync.dma_start(
                out=gw_l[:].rearrange("p (hc e) -> p hc e", e=E),
                in_=gwl[:].rearrange("(hc p) e -> p hc e", p=P),
            )


            warm_n = [0]

            def warm(dep_ap, kdim):
                n = min(dep_ap.shape[-1] if len(dep_ap.shape) == 2 else dep_ap.free_size(), 512)
                wps = psg.tile([8, n], f32, tag="mm3", name=f"warm{warm_n[0]}")
                warm_n[0] += 1
                nc.tensor.matmul(
                    out=wps[:],
                    lhsT=idt[0:kdim, 0:8],
                    rhs=dep_ap,
                    start=True, stop=True,
                )
            # ---- A. router: logitsT [8, 2048] = gw.T @ x.T, fp32 ----
            logitsT = sb.tile([E, T], f32, tag="logitsT")
            ps_l = [psg.tile([E, 512], f32, tag=f"mm{ng}", name=f"psl{ng}") for ng in range(4)]
            for hc in range(HC):
                xt_h = sbw.tile([P, T], bf16, tag="xth")
                nc.sync.dma_start(out=xt_h[:], in_=xth[hc * P:(hc + 1) * P, :])
                xt_l = sbw.tile([P, T], bf16, tag="xtl")
                nc.sync.dma_start(out=xt_l[:], in_=xtl[hc * P:(hc + 1) * P, :])
                for ng in range(4):
                    terms = [
                        (gw_h[:, hc * E:(hc + 1) * E], xt_h),
                        (gw_l[:, hc * E:(hc + 1) * E], xt_h),
                        (gw_h[:, hc * E:(hc + 1) * E], xt_l),
                    ]
                    for ti, (lw, xr) in enumerate(terms):
                        nc.tensor.matmul(
                            out=ps_l[ng][:],
                            lhsT=lw,
                            rhs=xr[:, ng * 512:(ng + 1) * 512],
                            start=(hc == 0 and ti == 0),
                            stop=(hc == HC - 1 and ti == 2),
                        )
            for ng in range(4):
                nc.vector.tensor_copy(
                    out=logitsT[:, ng * 512:(ng + 1) * 512], in_=ps_l[ng][:]
                )

            # ---- B. transpose logitsT -> l_all [128, (16, 8)] ----
            l_all = sb.tile([P, NT * E], f32, tag="l_all")
            for ci in range(NT):
                tp = psg.tile([P, E], f32, tag="mm3")
                nc.tensor.transpose(
                    out=tp[:],
                    in_=logitsT[:, ci * P:(ci + 1) * P],
                    identity=idt[0:E, 0:E],
                )
                nc.vector.tensor_copy(out=l_all[:, ci * E:(ci + 1) * E], in_=tp[:])

            # ---- C. top-2 + coef ----
            l3 = l_all[:].rearrange("p (t e) -> p t e", e=E)
            m1a = sb.tile([P, NT, 4], f32, tag="m1a")
            m2a = sb.tile([P, NT, 4], f32, tag="m2a")
            nc.vector.tensor_tensor(out=m1a[:], in0=l3[:, :, 0::2], in1=l3[:, :, 1::2], op=OP.max)
            nc.vector.tensor_tensor(out=m2a[:], in0=l3[:, :, 0::2], in1=l3[:, :, 1::2], op=OP.min)
            m1b = sb.tile([P, NT, 2], f32, tag="m1b")
            m2b = sb.tile([P, NT, 2], f32, tag="m2b")
            tmin = sb.tile([P, NT, 2], f32, tag="tmin")
            nc.vector.tensor_tensor(out=m1b[:], in0=m1a[:, :, 0::2], in1=m1a[:, :, 1::2], op=OP.max)
            nc.vector.tensor_tensor(out=tmin[:], in0=m1a[:, :, 0::2], in1=m1a[:, :, 1::2], op=OP.min)
            nc.vector.tensor_tensor(out=m2b[:], in0=m2a[:, :, 0::2], in1=m2a[:, :, 1::2], op=OP.max)
            nc.vector.tensor_tensor(out=m2b[:], in0=m2b[:], in1=tmin[:], op=OP.max)
            m1 = sb.tile([P, NT, 1], f32, tag="m1")
            m2 = sb.tile([P, NT, 1], f32, tag="m2")
            tmin2 = sb.tile([P, NT, 1], f32, tag="tmin2")
            nc.vector.tensor_tensor(out=m1[:], in0=m1b[:, :, 0:1], in1=m1b[:, :, 1:2], op=OP.max)
            nc.vector.tensor_tensor(out=tmin2[:], in0=m1b[:, :, 0:1], in1=m1b[:, :, 1:2], op=OP.min)
            nc.vector.tensor_tensor(out=m2[:], in0=m2b[:, :, 0:1], in1=m2b[:, :, 1:2], op=OP.max)
            nc.vector.tensor_tensor(out=m2[:], in0=m2[:], in1=tmin2[:], op=OP.max)

            warm(m1[:, :, 0], P)
            dq = sb.tile([P, NT], f32, tag="dq")
            nc.vector.tensor_sub(out=dq[:], in0=m2[:, :, 0], in1=m1[:, :, 0])
            q = sb.tile([P, NT], f32, tag="q")
            nc.scalar.activation(out=q[:], in_=dq[:], func=AF.Exp)
            s = sb.tile([P, NT], f32, tag="s")
            nc.vector.tensor_scalar_add(s[:], q[:], 1.0)
            wt1 = sb.tile([P, NT], f32, tag="wt1")
            nc.vector.reciprocal(wt1[:], s[:])
            wt2 = sb.tile([P, NT], f32, tag="wt2")
            nc.vector.tensor_mul(out=wt2[:], in0=q[:], in1=wt1[:])

            le_m = sb.tile([P, NT, E], f32, tag="lem")
            nc.vector.tensor_mul(
                out=le_m[:], in0=l3[:], in1=oh_sb[:].rearrange("p (t e) -> p t e", e=E)
            )
            le = sb.tile([P, NT], f32, tag="le")
            nc.vector.reduce_sum(
                out=le[:].rearrange("p (t o) -> p t o", o=1),
                in_=le_m[:],
                axis=mybir.AxisListType.X,
            )

            eq1 = sb.tile([P, NT], f32, tag="eq1")
            eq2 = sb.tile([P, NT], f32, tag="eq2")
            nc.vector.tensor_tensor(out=eq1[:], in0=le[:], in1=m1[:, :, 0], op=OP.is_equal)
            nc.vector.tensor_tensor(out=eq2[:], in0=le[:], in1=m2[:, :, 0], op=OP.is_equal)
            coef = sb.tile([P, NT], f32, tag="coef")
            t1 = sb.tile([P, NT], f32, tag="t1")
            nc.vector.tensor_mul(out=coef[:], in0=eq1[:], in1=wt1[:])
            nc.vector.tensor_mul(out=t1[:], in0=eq2[:], in1=wt2[:])
            nc.vector.tensor_add(out=coef[:], in0=coef[:], in1=t1[:])
            selm = sb.tile([P, NT], f32, tag="selm")
            nc.vector.tensor_add(out=selm[:], in0=eq1[:], in1=eq2[:])
            warm(coef[:], P)

            # ---- D. compaction ----
            # pack token id + gate coef in one fp32: v = tokid + min(coef,.999)/2
            # (coef recovered as 2*frac(v); idx as round-to-nearest of v)
            enc = sb.tile([P, NT], f32, tag="enc")
            tkp = sb.tile([P, NT], f32, tag="tkp")
            nc.vector.tensor_scalar_add(tkp[:], tk[:], 1.0)
            cfh = sb.tile([P, NT], f32, tag="cfh")
            nc.vector.tensor_scalar(cfh[:], coef[:], 0.999, 0.5, op0=OP.min, op1=OP.mult)
            nc.vector.tensor_add(out=enc[:], in0=tkp[:], in1=cfh[:])
            nc.vector.tensor_mul(out=enc[:], in0=enc[:], in1=selm[:])
            nc.vector.tensor_scalar_sub(enc[:], enc[:], 1.0)

            warm(enc[:], P)
            enc_t = sb.tile([NT, P], f32, tag="enc_t")
            tp1 = psg.tile([NT, P], f32, tag="mm3")
            nc.tensor.transpose(out=tp1[:], in_=enc[:], identity=idt[:])
            nc.vector.tensor_copy(out=enc_t[:], in_=tp1[:])
            warm(enc_t[:], 16)

            sg_v = sb.tile([16, P], f32, tag="sgv")
            nf1 = sb.tile([1, 1], u32, tag="nf1")
            nc.gpsimd.sparse_gather(out=sg_v[:], in_=enc_t[:], num_found=nf1[:])
            nc.sync.dma_start(out=o_cnt[:], in_=nf1[:])

            # valid-slot mask (sparse_gather tail is garbage on HW)
            nf_f = sb.tile([1, 1], f32, tag="nff")
            nc.vector.tensor_copy(out=nf_f[:], in_=nf1[:])
            ones16 = sb.tile([1, 16], f32, tag="ones16")
            nc.vector.memset(ones16[:], 1.0)
            nf_b_ps = psg.tile([16, 1], f32, tag="mm3")
            nc.tensor.matmul(out=nf_b_ps[:], lhsT=ones16[:], rhs=nf_f[:], start=True, stop=True)
            nf_b = sb.tile([16, 1], f32, tag="nfbs")
            nc.vector.tensor_copy(out=nf_b[:], in_=nf_b_ps[:])
            slot_mask = sb.tile([16, cf], u8, tag="slotm")
            nc.vector.tensor_tensor(
                out=slot_mask[:], in0=slotg[:],
                in1=nf_b[:].to_broadcast([16, cf]), op=OP.is_lt,
            )
            v_f = sb.tile([16, cf], f32, tag="vf")
            nc.vector.memset(v_f[:], 0.0)
            nc.vector.copy_predicated(out=v_f[:], mask=slot_mask[:], data=sg_v[:, 0:cf])
            warm(v_f[:], 16)
            idx_i = sb.tile([16, cf], i32, tag="idxi")
            nc.vector.tensor_copy(out=idx_i[:], in_=v_f[:])
            nc.sync.dma_start(out=o_idx[:].rearrange("(f p) -> p f", p=16), in_=idx_i[:])

            # free-major relayout of packed values through DRAM
            nc.sync.dma_start(out=d_cf[:].rearrange("(f p) -> p f", p=16), in_=v_f[:])
            nft = cap // P           # full 128-token gather tiles
            rem = cap - nft * P      # remainder tile rows
            v_sb = sb.tile([P, nft], f32, tag="vsb")
            nc.sync.dma_start(
                out=v_sb[:], in_=d_cf[0:nft * P].rearrange("(k p) -> p k", p=P)
            )
            idx_sb = sb.tile([P, nft], i32, tag="idxsb")
            nc.vector.tensor_copy(out=idx_sb[:], in_=v_sb[:])
            warm(v_sb[:], P)
            if rem:
                v_sb2 = sb.tile([rem, 1], f32, tag="vsb2")
                nc.sync.dma_start(
                    out=v_sb2[:],
                    in_=d_cf[nft * P:cap].rearrange("(p o) -> p o", o=1),
                )
                idx_sb2 = sb.tile([rem, 1], i32, tag="idxsb2")
                nc.vector.tensor_copy(out=idx_sb2[:], in_=v_sb2[:])
            vrow = sb.tile([1, cap], f32, tag="vrow")
            nc.sync.dma_start(out=vrow[:], in_=d_cf[:].rearrange("(o c) -> o c", o=1))
            vrow_i = sb.tile([1, cap], i32, tag="vrowi")
            nc.vector.tensor_copy(out=vrow_i[:], in_=vrow[:])
            vrow_f = sb.tile([1, cap], f32, tag="vrowf")
            nc.vector.tensor_copy(out=vrow_f[:], in_=vrow_i[:])
            cf_row = sb.tile([1, cap], f32, tag="cfrow")
            nc.vector.tensor_sub(out=cf_row[:], in0=vrow[:], in1=vrow_f[:])
            nc.vector.tensor_scalar_mul(cf_row[:], cf_row[:], 2.0)
            warm(cf_row[:, 0:512], 1)

            # ---- F. coef broadcast [128, cap] ----
            onesP = sb.tile([1, P], f32, tag="onesP")
            nc.vector.memset(onesP[:], 1.0)
            cbc = sb.tile([P, cap], f32, tag="cbc")
            for g in range(ngrp):
                cb_ps = psg.tile([P, grp], f32, tag="mm2")
                nc.tensor.matmul(
                    out=cb_ps[:], lhsT=onesP[:],
                    rhs=cf_row[:, g * grp:(g + 1) * grp], start=True, stop=True,
                )
                nc.vector.tensor_copy(out=cbc[:, g * grp:(g + 1) * grp], in_=cb_ps[:])

            # ---- E. gather selected token rows + transpose to [H, cap] ----
            xgT = [sb.tile([P, cap], bf16, tag=f"xgT{hc}", name=f"xgT{hc}") for hc in range(HC)]
            gtiles = [(k * P, P) for k in range(nft)] + ([(nft * P, rem)] if rem else [])
            for gi, (goff, gn) in enumerate(gtiles):
                off_ap = idx_sb[:, gi:gi + 1] if gn == P else idx_sb2[:, 0:1]
                xg = sbw.tile([gn, H], bf16, tag="xg", name=f"xg{gi}", bufs=3)
                nc.gpsimd.indirect_dma_start(
                    out=xg[:], out_offset=None,
                    in_=x[:],
                    in_offset=bass.IndirectOffsetOnAxis(ap=off_ap, axis=0),
                )
                for hc in range(HC):
                    tpx = psg.tile([P, gn], bf16, tag="mm3", name=f"xtr{gi}{hc}")
                    nc.tensor.transpose(
                        out=tpx[:], in_=xg[:, hc * P:(hc + 1) * P],
                        identity=idtb[0:gn, 0:gn],
                    )
                    nc.vector.tensor_copy(
                        out=xgT[hc][:, goff:goff + gn], in_=tpx[:]
                    )

            # ---- G. h1 = x@w1, h3 = x@w3 (transposed), fused silu*mul ----
            actT = [sb.tile([P, cap], bf16, tag=f"actT{ic}", name=f"actT{ic}") for ic in range(IC)]
            for ic in range(IC):
                w1_sl = sbw.tile([P, H], bf16, tag="w1sl", bufs=4)
                nc.sync.dma_start(
                    out=w1_sl[:].rearrange("p (hc i) -> p hc i", i=P),
                    in_=w1[:, ic * P:(ic + 1) * P].rearrange("(hc p) i -> p hc i", p=P),
                )
                w3_sl = sbw.tile([P, H], bf16, tag="w3sl", bufs=4)
                nc.sync.dma_start(
                    out=w3_sl[:].rearrange("p (hc i) -> p hc i", i=P),
                    in_=w3[:, ic * P:(ic + 1) * P].rearrange("(hc p) i -> p hc i", p=P),
                )
                for g in range(ngrp):
                    gs = slice(g * grp, (g + 1) * grp)
                    ps1 = psg.tile([P, grp], f32, tag="mm0")
                    ps3 = psg.tile([P, grp], f32, tag="mm1")
                    for hc in range(HC):
                        nc.tensor.matmul(
                            out=ps1[:],
                            lhsT=w1_sl[:, hc * P:(hc + 1) * P],
                            rhs=xgT[hc][:, gs],
                            start=(hc == 0), stop=(hc == HC - 1),
                        )
                    for hc in range(HC):
                        nc.tensor.matmul(
                            out=ps3[:],
                            lhsT=w3_sl[:, hc * P:(hc + 1) * P],
                            rhs=xgT[hc][:, gs],
                            start=(hc == 0), stop=(hc == HC - 1),
                        )
                    sl = sbw.tile([P, grp], f32, tag="silu")
                    nc.scalar.activation(out=sl[:], in_=ps1[:], func=AF.Silu)
                    nc.vector.tensor_mul(out=actT[ic][:, gs], in0=sl[:], in1=ps3[:])

            # ---- H. yT = (act @ w2).T * coef ----
            for hc in range(HC):
                w2_sl = sbw.tile([P, II], bf16, tag="w2sl", bufs=3)
                nc.sync.dma_start(
                    out=w2_sl[:].rearrange("p (ic h) -> p ic h", h=P),
                    in_=w2[:, hc * P:(hc + 1) * P].rearrange("(ic p) h -> p ic h", p=P),
                )
                for g in range(ngrp):
                    gs = slice(g * grp, (g + 1) * grp)
                    pso = psg.tile([P, grp], f32, tag="mm2")
                    for ic in range(IC):
                        nc.tensor.matmul(
                            out=pso[:],
                            lhsT=w2_sl[:, ic * P:(ic + 1) * P],
                            rhs=actT[ic][:, gs],
                            start=(ic == 0), stop=(ic == IC - 1),
                        )
                    yt_sb = sbw.tile([P, grp], f32, tag="yt")
                    nc.vector.tensor_mul(out=yt_sb[:], in0=pso[:], in1=cbc[:, gs])
                    nc.sync.dma_start(
                        out=o_yt[hc * P:(hc + 1) * P, gs], in_=yt_sb[:]
                    )

    nc.compile()
    return nc


def _get_built(cap):
    if cap not in _build_cache:
        _build_cache[cap] = _build(cap)
    return _build_cache[cap]


def _run(cap, hs, gate_w, w1s, w2s, w3s, trace=False):
    import ml_dtypes
    from concourse.bass_utils import run_bass_kernel_spmd

    nc = _get_built(cap)

    bf = ml_dtypes.bfloat16
    x_hi = hs.astype(bf)
    x_lo = (hs - x_hi.astype(np.float32)).astype(bf)
    xth_np = np.ascontiguousarray(x_hi.T)
    xtl_np = np.ascontiguousarray(x_lo.T)
    gw_hi = gate_w.astype(bf)
    gw_lo = (gate_w - gw_hi.astype(np.float32)).astype(bf)
    x_bf = np.ascontiguousarray(x_hi)
    oh_base = np.zeros((P, NT, E), np.float32)
    tokid_np = (np.arange(NT)[None, :] * P + np.arange(P)[:, None]).astype(np.float32)
    slotg_np = (np.arange(cap // 16)[None, :] * 16 + np.arange(16)[:, None]).astype(np.float32)
    ident_np = np.eye(P, dtype=np.float32)

    in_maps = []
    for c in range(NCORES):
        oh_c = oh_base.copy()
        oh_c[:, :, c] = 1.0
        in_maps.append({
            "xth": xth_np,
            "xtl": xtl_np,
            "x": x_bf,
            "gwh": gw_hi,
            "gwl": gw_lo,
            "w1": np.ascontiguousarray(w1s[c].astype(bf)),
            "w3": np.ascontiguousarray(w3s[c].astype(bf)),
            "w2": np.ascontiguousarray(w2s[c].astype(bf)),
            "oh": oh_c.reshape(P, NT * E),
            "tokid": tokid_np,
            "slotg": slotg_np,
            "ident": ident_np,
        })

    res = run_bass_kernel_spmd(nc, in_maps, list(range(NCORES)), trace=trace)
    return res


def kernel(hidden_states, gate_w, w1s, w2s, w3s, _trace=False, _cap=608):
    hs = np.ascontiguousarray(np.asarray(hidden_states, dtype=np.float32))
    gate_w = np.ascontiguousarray(np.asarray(gate_w, dtype=np.float32))
    w1s = np.asarray(w1s, dtype=np.float32)
    w2s = np.asarray(w2s, dtype=np.float32)
    w3s = np.asarray(w3s, dtype=np.float32)

    cap = _cap
    while True:
        res = _run(cap, hs, gate_w, w1s, w2s, w3s, trace=_trace)
        counts = [int(res.results[c]["o_cnt"].ravel()[0]) for c in range(NCORES)]
        if max(counts) <= cap:
            break
        # capacity overflow (won't happen for sane routing): rebuild bigger
        cap = 2048 if max(counts) > 1024 else 1024

    out = np.zeros((T, H), dtype=np.float32)
    for c in range(NCORES):
        r = res.results[c]
        cnt = counts[c]
        idx = r["o_idx"][:cnt]
        y = np.ascontiguousarray(r["o_yt"].T[:cnt])
        out[idx] += y
    kernel._last_results = res
    return out
